# revision 10
# baseline (speedup 1.0000x reference)
"""GAT kernel v2 for Trainium2, 8-core SPMD.

Changes vs baseline (kernel.py):
  - Stage A: one PE matmul per tile against an augmented weight matrix
    [F_IN, 136] whose last 8 columns are W^T a_src / W^T a_dst (built on
    device), yielding h | e_src | e_dst in one PSUM tile. Removes the
    per-tile mul+reduce DVE ops.
  - Stage C: gathers batched G=4 tiles per (batch, window) call (4x fewer
    Pool-engine descriptor-gen launches). Scores use a unified per-batch
    buffer [P, G, K, Jt] with unit-stride inner dims. exp(s-m) is expanded
    over the feature dim on the Activation engine so the weighted multiply
    runs 2-byte unit-stride (DVE 2x mode). The slot reduction is a chain of
    small unit-stride fp16 adds (TensorReduce has no fast mode).
"""
import sys

if "/opt/trn_rl_repo" not in sys.path:
    sys.path.insert(0, "/opt/trn_rl_repo")

import hashlib
import numpy as np

N, DEG, K, F_IN, F_OUT = 100000, 16, 4, 128, 32
KF = K * F_OUT            # 128
N_CORES = 8
S = N // N_CORES          # 12500
P = 128
NT = (S + P - 1) // P     # 98
SP = NT * P               # 12544
NTAB = N_CORES * SP       # 100352
EL = 256                  # bf16 elements per table row (512B)
EOFF = 128                # h at [0,128); e_dst hi bf16 [128,132); lo [132,136)
BOUNDS = (0, 2 * SP, 4 * SP, 6 * SP, NTAB)   # shard-pair aligned (25088)
NW = 4
DUMMY = (S, 2 * SP + S, 4 * SP + S, 6 * SP + S)
NEG_SLOPE = 0.01
NEG_BIG = -1.0e30
G = 4                     # tiles per stage-C batch


def host_plan(nbr, g_batch=G):
    """Node ordering per core, per-batch-uniform J, batched idx buffers."""
    nbr = np.asarray(nbr).astype(np.int64)
    src_core = nbr // S
    win = src_core // 2                                     # [N, DEG] in 0..3
    orders = []
    cnts = []
    for c in range(N_CORES):
        w = win[c * S:(c + 1) * S]
        cnt = np.stack([(w == q).sum(1) for q in range(NW)], 1)  # [S,NW]
        order = np.lexsort((cnt[:, 3], cnt[:, 2], cnt[:, 1], cnt[:, 0]))
        orders.append(order)
        cnts.append(cnt)
    inv = np.empty(N, np.int64)
    for c in range(N_CORES):
        inv[c * S + orders[c]] = np.arange(S)
    rows = (src_core * SP + inv[nbr]).astype(np.int32)
    percore = []
    Js = np.zeros((N_CORES, NT, NW), np.int64)
    for c in range(N_CORES):
        r = np.sort(rows[c * S:(c + 1) * S], axis=1)[orders[c]]  # [S,16]
        cnt = cnts[c][orders[c]]
        rs = np.concatenate([r, np.zeros((SP - S, DEG), np.int32)])
        cs = np.concatenate([cnt, np.zeros((SP - S, NW), np.int64)])
        start = np.concatenate(
            [np.zeros((SP, 1), np.int64), np.cumsum(cs, 1)[:, :-1]], 1)
        percore.append((rs, cs, start))
        Js[c] = cs.reshape(NT, P, NW).max(1)
    J = Js.max(axis=0)                                      # [NT, NW]

    # batches with uniform per-window counts; greedy size so g*jt <= SLOT_CAP
    SLOT_CAP = 80
    batches = []
    t0 = 0
    while t0 < NT:
        g = 1
        jws = J[t0:t0 + 1].max(axis=0).astype(np.int64)
        while (t0 + g < NT and g < g_batch):
            jws2 = np.maximum(jws, J[t0 + g])
            if (g + 1) * int(jws2.sum()) > SLOT_CAP:
                break
            jws = jws2
            g += 1
        offs = np.concatenate([[0], np.cumsum(jws)[:-1]])
        batches.append({"t0": t0, "g": g, "jws": jws, "offs": offs,
                        "jt": int(jws.sum())})
        t0 += g

    idxbufs = []
    for c in range(N_CORES):
        rs, cs, start = percore[c]
        segs = []
        for b in batches:
            t0, g, jws = b["t0"], b["g"], b["jws"]
            for w in range(NW):
                jw = int(jws[w])
                if jw == 0:
                    continue
                vals = np.empty((g, jw, P), np.int64)
                jj = np.arange(jw)[None, :]
                for tr in range(g):
                    t = t0 + tr
                    rt = rs[t * P:(t + 1) * P]
                    ct = cs[t * P:(t + 1) * P]
                    st = start[t * P:(t + 1) * P]
                    take = st[:, w:w + 1] + jj
                    valid = jj < ct[:, w:w + 1]
                    v = np.where(
                        valid,
                        np.take_along_axis(
                            rt, np.minimum(take, DEG - 1).astype(np.int64), 1),
                        DUMMY[w]).astype(np.int64) - BOUNDS[w]
                    vals[tr] = v.T
                lin = vals.reshape(-1)                       # (t, d, p)
                segs.append(lin.reshape(-1, 16).T.astype(np.int16))
        buf16 = np.concatenate(segs, axis=1)
        idxbufs.append(np.ascontiguousarray(np.tile(buf16, (8, 1))))
    return J, orders, batches, idxbufs


def build_nc(J, batches, ctot, n_cores=N_CORES, nt=NT, distributed=True,
             one_queue=False):
    from contextlib import ExitStack

    import concourse.bass as bass
    import concourse.tile as tile
    from concourse import bacc, mybir
    from concourse.masks import make_identity

    f32 = mybir.dt.float32
    bf16 = mybir.dt.bfloat16
    f16 = mybir.dt.float16
    i16 = mybir.dt.int16
    sp = nt * P

    nc = bacc.Bacc("TRN2", target_bir_lowering=False, debug=False,
                   num_devices=n_cores,
                   num_swdge_queues=1 if one_queue else 4)

    xs = nc.dram_tensor("xs", [sp, F_IN], f32, kind="ExternalInput")
    wt = nc.dram_tensor("wt", [F_IN, KF], f32, kind="ExternalInput")
    wtT = nc.dram_tensor("wtT", [KF, F_IN], f32, kind="ExternalInput")
    avT = nc.dram_tensor("avT", [KF, 2], f32, kind="ExternalInput")
    mask8 = nc.dram_tensor("mask8", [P, 8], f32, kind="ExternalInput")
    idxin = nc.dram_tensor("idxin", [P, ctot], i16, kind="ExternalInput")
    padfill = nc.dram_tensor("padfill", [sp - S if sp > S else 1, 8], bf16,
                             kind="ExternalInput")
    out = nc.dram_tensor("out", [sp, KF], f32, kind="ExternalOutput")

    he_shard = nc.dram_tensor("he_shard", [sp, EL], bf16, kind="Internal")
    he_full = nc.dram_tensor("he_full", [NTAB, EL], bf16, kind="Internal",
                             addr_space="Shared" if distributed else "Local")

    with tile.TileContext(nc) as tc, ExitStack() as ctx:
        consts = ctx.enter_context(tc.tile_pool(name="consts", bufs=1))
        sa = ctx.enter_context(tc.tile_pool(name="sa", bufs=3))
        sa_ps = ctx.enter_context(tc.tile_pool(name="sa_ps", bufs=2, space="PSUM"))
        sg = ctx.enter_context(tc.tile_pool(name="sg", bufs=2))
        saf = ctx.enter_context(tc.tile_pool(name="saf", bufs=2))
        sidx = ctx.enter_context(tc.tile_pool(name="sidx", bufs=2))
        sc = ctx.enter_context(tc.tile_pool(name="sc", bufs=2))

        ident = consts.tile([P, P], f32)
        make_identity(nc, ident[:])
        wt_aug = consts.tile([F_IN, KF + 8], f32)
        nc.sync.dma_start(wt_aug[:, 0:KF], wt.ap())
        wtT_sb = consts.tile([KF, F_IN], f32)
        nc.sync.dma_start(wtT_sb[:], wtT.ap())
        avT_sb = consts.tile([P, 2], f32)
        nc.sync.dma_start(avT_sb[:], avT.ap())
        mask8_sb = consts.tile([P, 8], f32)
        nc.sync.dma_start(mask8_sb[:], mask8.ap())
        es_sb = consts.tile([P, nt * K], f32)

        # ---- setup: wt_aug[:, 128:136] = [W^T a_src | W^T a_dst] ----
        adiag = consts.tile([P, 8], f32)
        nc.vector.tensor_mul(adiag[:, 0:4], mask8_sb[:, 0:4],
                             avT_sb[:, 0:1].to_broadcast([P, 4]))
        nc.vector.tensor_mul(adiag[:, 4:8], mask8_sb[:, 4:8],
                             avT_sb[:, 1:2].to_broadcast([P, 4]))
        ps8 = sa_ps.tile([8, KF], f32, tag="ps8")
        nc.tensor.matmul(ps8[:], lhsT=adiag[:], rhs=wtT_sb[:],
                         start=True, stop=True)
        ps8_sb = consts.tile([8, KF], f32)
        nc.vector.tensor_copy(ps8_sb[:], ps8[:])
        psT = sa_ps.tile([P, 8], f32, tag="psT")
        nc.tensor.transpose(out=psT[:], in_=ps8_sb[:], identity=ident[0:8, 0:8])
        nc.vector.tensor_copy(wt_aug[:, KF:KF + 8], psT[:])

        # ---- Stage A ----
        for t in range(nt):
            x_t = sa.tile([P, F_IN], f32, tag="x")
            nc.sync.dma_start(x_t[:], xs.ap()[t * P:(t + 1) * P, :])
            xt_ps = sa_ps.tile([P, P], f32, tag="xt")
            nc.tensor.transpose(out=xt_ps[:], in_=x_t[:], identity=ident[:])
            xt_sb = sa.tile([P, P], f32, tag="xt_sb")
            nc.scalar.copy(xt_sb[:], xt_ps[:])
            h_ps = sa_ps.tile([P, KF + 8], f32, tag="h")
            nc.tensor.matmul(h_ps[:], lhsT=xt_sb[:], rhs=wt_aug[:],
                             start=True, stop=True)
            he_t = sa.tile([P, EL], bf16, tag="he")
            nc.scalar.copy(he_t[:, 0:KF], h_ps[:, 0:KF])
            nc.vector.tensor_copy(es_sb[:, t * K:(t + 1) * K],
                                  h_ps[:, KF:KF + 4])
            nc.vector.tensor_copy(he_t[:, EOFF:EOFF + 4], h_ps[:, KF + 4:KF + 8])
            ehi32 = sa.tile([P, K], f32, tag="ehi32")
            nc.vector.tensor_copy(ehi32[:], he_t[:, EOFF:EOFF + 4])
            elo = sa.tile([P, K], f32, tag="elo")
            nc.vector.tensor_sub(elo[:], h_ps[:, KF + 4:KF + 8], ehi32[:])
            nc.vector.tensor_copy(he_t[:, EOFF + 4:EOFF + 8], elo[:])
            nc.sync.dma_start(he_shard.ap()[t * P:(t + 1) * P, 0:EOFF + 8],
                              he_t[:, 0:EOFF + 8])
        npad = sp - S
        if npad > 0:
            pf = consts.tile([npad, 8], bf16)
            nc.sync.dma_start(pf[:], padfill.ap())
            nc.sync.dma_start(he_shard.ap()[S:sp, EOFF:EOFF + 8], pf[:])
            zpad = consts.tile([npad, KF], bf16)
            nc.vector.memset(zpad[:], 0.0)
            nc.sync.dma_start(he_shard.ap()[S:sp, 0:KF], zpad[:])

        # ---- Stage B ----
        if distributed:
            nc.gpsimd.collective_compute(
                "AllGather", mybir.AluOpType.bypass,
                replica_groups=[list(range(n_cores))],
                ins=[he_shard.ap()], outs=[he_full.ap()])
        else:
            for t in range(nt):
                cp = sa.tile([P, EL], bf16, tag="cp")
                nc.sync.dma_start(cp[:], he_shard.ap()[t * P:(t + 1) * P, :])
                nc.sync.dma_start(he_full.ap()[t * P:(t + 1) * P, :], cp[:])

        # ---- Stage C ----
        coff = 0
        ncall = 0
        for b in batches:
            t0, g, jws, offs, jt = b["t0"], b["g"], b["jws"], b["offs"], b["jt"]
            gjt = g * jt
            gbuf = sg.tile([P, gjt * EL], bf16, tag="g")
            bcols = sum(g * int(jw) * 8 for jw in jws if jw)
            idx_sb = sidx.tile([P, bcols], i16, tag="idx")
            nc.sync.dma_start(idx_sb[:], idxin.ap()[:, coff:coff + bcols])
            bcoff = 0
            # gathers: region for window w at slot offset g*offs[w]
            for w in range(NW):
                jw = int(jws[w])
                if jw == 0:
                    continue
                nidx = g * jw * P
                roff = g * int(offs[w])
                g3 = gbuf[:, roff * EL:(roff + g * jw) * EL].rearrange(
                    "p (s e) -> p s e", e=EL)
                nc.gpsimd.dma_gather(
                    out_ap=g3,
                    in_ap=he_full.ap()[BOUNDS[w]:BOUNDS[w + 1], :],
                    idxs_ap=idx_sb[:, bcoff:bcoff + g * jw * 8],
                    num_idxs=nidx, num_idxs_reg=nidx, elem_size=EL,
                    single_packet=False,
                    queue_num=0 if one_queue else ncall % 4)
                ncall += 1
                bcoff += g * jw * 8
            coff += bcols

            # scores: s1[p, g, k, jt] (k-major, slot inner)
            s1 = sc.tile([P, g * K * jt], f32, tag="s1")
            s1v = s1[:].rearrange("p (t k d) -> p t k d", k=K, d=jt)
            for w in range(NW):
                jw = int(jws[w])
                if jw == 0:
                    continue
                roff = g * int(offs[w])
                gw = gbuf[:, roff * EL:(roff + g * jw) * EL].rearrange(
                    "p (t d e) -> p t d e", d=jw, e=EL)
                ehi = gw[:, :, :, EOFF:EOFF + 4].rearrange("p t d k -> p t k d")
                elo = gw[:, :, :, EOFF + 4:EOFF + 8].rearrange(
                    "p t d k -> p t k d")
                o0 = int(offs[w])
                nc.vector.tensor_add(s1v[:, :, :, o0:o0 + jw], ehi, elo)
            esrc = es_sb[:, t0 * K:(t0 + g) * K].rearrange(
                "p (t k) -> p t k", k=K).unsqueeze(-1).to_broadcast(
                [P, g, K, jt])
            s2 = sc.tile([P, g * K * jt], f32, tag="s2")
            s2v = s2[:].rearrange("p (t k d) -> p t k d", k=K, d=jt)
            nc.vector.tensor_add(s2v, s1v, esrc)
            s3 = sc.tile([P, g * K * jt], f32, tag="s3")
            nc.vector.scalar_tensor_tensor(
                s3[:], s2[:], NEG_SLOPE, s2[:],
                op0=mybir.AluOpType.mult, op1=mybir.AluOpType.max)
            s3v = s3[:].rearrange("p (t k d) -> p t k d", k=K, d=jt)
            m = sc.tile([P, g * K], f32, tag="m")
            nc.vector.reduce_max(
                m[:], s3[:].rearrange("p (tk d) -> p tk d", d=jt),
                axis=mybir.AxisListType.X)
            s4 = sc.tile([P, g * K * jt], f32, tag="s4")
            s4v = s4[:].rearrange("p (t k d) -> p t k d", k=K, d=jt)
            nc.vector.tensor_sub(
                s4v, s3v,
                m[:].rearrange("p (t k) -> p t k", k=K)
                   .unsqueeze(-1).to_broadcast([P, g, K, jt]))
            pr = sc.tile([P, g * K * jt], f32, tag="pr")
            nc.scalar.activation(pr[:], s4[:], mybir.ActivationFunctionType.Exp)
            z = sc.tile([P, g * K], f32, tag="z")
            nc.vector.reduce_sum(
                z[:], pr[:].rearrange("p (tk d) -> p tk d", d=jt),
                axis=mybir.AxisListType.X)
            rz = sc.tile([P, g * K], f32, tag="rz")
            nc.vector.reciprocal(rz[:], z[:])

            # expanded exp over features (ACT), weighted mult (DVE 2x),
            # slot-sum via f16 adds (ping-pong accumulators)
            s4q = s4[:].rearrange("p (t k d) -> p t k d", k=K, d=jt)

            def acc_tile(tag):
                tt = sc.tile([P, g * KF], f16, tag=tag)
                return tt[:].rearrange("p (t o kf) -> p t o kf", o=1, kf=KF)

            pp = [0]

            def pair_sum(v0, v1):
                dst = acc_tile(f"pp{pp[0] % 2}")
                pp[0] += 1
                nc.vector.tensor_add(dst, v0, v1)
                return dst

            u = None
            for w in range(NW):
                jw = int(jws[w])
                if jw == 0:
                    continue
                roff = g * int(offs[w])
                o0 = int(offs[w])
                af = saf.tile([P, g * jw * KF], f16, tag="af")
                # reorder (t,k,d) -> (t,d,k) compact so the f-broadcast
                # expansion is a 3D free pattern
                cw = sc.tile([P, g * jw * K], f32, tag="cw")
                nc.vector.tensor_copy(
                    cw[:].rearrange("p (t d k) -> p t d k", d=jw, k=K),
                    s4q[:, :, :, o0:o0 + jw].rearrange("p t k d -> p t d k"))
                afv = af[:].rearrange("p (s k f) -> p s k f", k=K, f=F_OUT)
                sin = cw[:].rearrange("p (s k) -> p s k", k=K).unsqueeze(
                    -1).to_broadcast([P, g * jw, K, F_OUT])
                nc.scalar.activation(afv, sin,
                                     mybir.ActivationFunctionType.Exp)
                gw = gbuf[:, roff * EL:(roff + g * jw) * EL].rearrange(
                    "p (s e) -> p s e", e=EL)
                # weighted mult in place over the gather rows' h columns
                # (dead after this op)
                nc.vector.tensor_mul(
                    gw[:, :, 0:KF],
                    gw[:, :, 0:KF], af[:].rearrange("p (s kf) -> p s kf",
                                                    kf=KF))
                # sum over d within window: pairwise tree folds. Level 1
                # folds the bf16 products into the (now dead) af buffer in
                # f16; later levels fold af in place.
                wgv = gw.rearrange("p (t d) e -> p t d e", d=jw)[:, :, :, 0:KF]
                afw = af[:].rearrange("p (t d kf) -> p t d kf", d=jw, kf=KF)
                if jw == 1:
                    uw = wgv[:, :, 0:1, :]
                elif jw <= 3:
                    uw = pair_sum(wgv[:, :, 0:1, :], wgv[:, :, 1:2, :])
                    for d in range(2, jw):
                        uw = pair_sum(uw, wgv[:, :, d:d + 1, :])
                else:
                    n = jw
                    h = n // 2
                    nc.vector.tensor_add(afw[:, :, 0:h, :],
                                         wgv[:, :, 0:h, :],
                                         wgv[:, :, n - h:n, :])
                    if n % 2:
                        nc.vector.tensor_copy(afw[:, :, h:h + 1, :],
                                              wgv[:, :, h:h + 1, :])
                    n = n - h
                    while n > 1:
                        h = n // 2
                        nc.vector.tensor_add(afw[:, :, 0:h, :],
                                             afw[:, :, 0:h, :],
                                             afw[:, :, n - h:n, :])
                        n = n - h
                    uw = afw[:, :, 0:1, :]
                u = uw if u is None else pair_sum(u, uw)

            # out = ELU(u * rz)
            o = sc.tile([P, g * KF], f32, tag="o")
            nc.vector.tensor_mul(
                o[:].rearrange("p (t k f) -> p t k f", k=K, f=F_OUT),
                u.rearrange("p t o (k f) -> p (t o) k f", f=F_OUT),
                rz[:].rearrange("p (t k) -> p t k", k=K)
                    .unsqueeze(-1).to_broadcast([P, g, K, F_OUT]))
            t1 = sc.tile([P, g * KF], f32, tag="t1")
            nc.vector.tensor_scalar_min(t1[:], o[:], 0.0)
            e1 = sc.tile([P, g * KF], f32, tag="e1")
            nc.scalar.activation(e1[:], t1[:], mybir.ActivationFunctionType.Exp)
            r = sc.tile([P, g * KF], f32, tag="r")
            nc.vector.tensor_scalar_max(r[:], o[:], 0.0)
            ot = sc.tile([P, g * KF], f32, tag="ot")
            nc.vector.scalar_tensor_tensor(
                ot[:], e1[:], -1.0, r[:],
                op0=mybir.AluOpType.add, op1=mybir.AluOpType.add)
            for tr in range(g):
                t = t0 + tr
                nc.sync.dma_start(out.ap()[t * P:(t + 1) * P, :],
                                  ot[:, tr * KF:(tr + 1) * KF])

    nc.compile()
    return nc


def prep_inputs(X, W, a, nbr):
    X = np.asarray(X, dtype=np.float32)
    W = np.asarray(W, dtype=np.float32)
    a = np.asarray(a, dtype=np.float32)
    J, orders, batches, idxbufs = host_plan(nbr)
    ctot = idxbufs[0].shape[1]
    wt = np.ascontiguousarray(W.transpose(2, 0, 1).reshape(F_IN, KF))
    wtT = np.ascontiguousarray(W.reshape(KF, F_IN))
    a_src = a[:, 0, :F_OUT].reshape(KF)
    a_dst = a[:, 0, F_OUT:].reshape(KF)
    avT = np.ascontiguousarray(np.stack([a_src, a_dst], axis=1))
    mask8 = np.zeros((P, 8), np.float32)
    for k in range(K):
        mask8[k * F_OUT:(k + 1) * F_OUT, k] = 1.0
        mask8[k * F_OUT:(k + 1) * F_OUT, 4 + k] = 1.0
    import ml_dtypes
    pf = np.full((max(SP - S, 1), 8), NEG_BIG, dtype=ml_dtypes.bfloat16)
    in_maps = []
    for c in range(N_CORES):
        xsb = np.zeros((SP, F_IN), dtype=np.float32)
        xsb[:S] = X[c * S:(c + 1) * S][orders[c]]
        in_maps.append({"xs": xsb, "wt": wt, "wtT": wtT, "avT": avT,
                        "mask8": mask8, "idxin": idxbufs[c], "padfill": pf})
    return J, orders, batches, ctot, in_maps


_NC_CACHE = {}


def kernel(X, W, a, nbr):
    from concourse.bass_utils import run_bass_kernel_spmd

    J, orders, batches, ctot, in_maps = prep_inputs(X, W, a, nbr)
    key = hashlib.sha1(J.tobytes()).hexdigest()
    if key not in _NC_CACHE:
        _NC_CACHE[key] = build_nc(J, batches, ctot)
    nc = _NC_CACHE[key]
    res = run_bass_kernel_spmd(nc, in_maps, core_ids=list(range(N_CORES)))
    out = np.empty((N, KF), dtype=np.float32)
    for c in range(N_CORES):
        out[c * S + orders[c]] = res.results[c]["out"][:S]
    return out
